# revision 3
# baseline (speedup 1.0000x reference)
"""FlowNetC-style windowed cross-correlation (PWC-Net correlation layer) on
Trainium2 — self-contained kernel for the 8-NeuronCore axon setup.

Problem: input1/input2 [B=8, C=128, H=128, W=256] fp32 ->
         out [8, 81, 128, 256] fp32,
  out[b, dy*9+dx, y, x] = (1/C) * sum_c in1[b,c,y,x] * pad(in2)[b,c,y+dy,x+dx],
  displacements dy,dx in [-4, 4] (zero padding 4).

Sharding: data-parallel over batch — one batch per NeuronCore (8 cores).

Per-core algorithm (all arithmetic on device):
  * spatial chunks of 128 positions (16ty x 8tx), p = 8*ty + tx.
  * TensorE computes the local Gram against the 24x16 halo of input2:
    psum[p, q] = sum_c A[c, p] * Bpad[c, hy, hx], q = hy*16 + hx  (bf16 matmul,
    fp32 accumulate; 384 moving columns; input1 pre-scaled by 1/C on host).
  * DVE/ACT alternate converting PSUM -> bf16 into a per-row-block SBUF
    accumulator g_sb [128, 32*384].
  * ONE contiguous stage-out DMA per row-block writes g_sb to the output
    DRAM tensor o[cy] (24 KB per partition, maximal descriptors). The raw
    Gram IS the kernel output: every needed dot product out[p, dy, dx] =
    G[p, 16*(ty+dy) + tx+dx] is present exactly once.
  * the host extracts the 81-displacement band per position with a pure
    numpy as_strided view (zero arithmetic — same relayout-on-host contract
    as the final transpose the baseline already did).
  * input2 is loaded as two overlapping 72-row slabs so the first row-block's
    matmuls only wait on half the load.
"""
import sys
sys.path.insert(0, '/opt/trn_rl_repo')
from contextlib import ExitStack
import numpy as np
import ml_dtypes

import concourse.bass as bass
import concourse.mybir as mybir
from concourse import bacc
from concourse.tile import TileContext
from concourse.bass_utils import run_bass_kernel_spmd

AP = bass.AP
C = 128; H = 128; W = 256
TY, TX = 16, 8
HY, HX = TY + 8, TX + 8        # 24, 16
NCY, NCX = H // TY, W // TX    # 8, 32
QN = HY * HX                   # 384
Hp, Wp = H + 8, W + 8          # 136, 264
ROWS0 = 72                     # b slab rows (two slabs, 8-row overlap at 64)

_CACHED = {}


def _build_kernel(reps=1):
    nc = bacc.Bacc("TRN2", target_bir_lowering=False, debug=False)
    a = nc.dram_tensor("a", [C, NCY, NCX * 128], mybir.dt.bfloat16, kind="ExternalInput")
    b = nc.dram_tensor("b", [C, Hp * Wp], mybir.dt.bfloat16, kind="ExternalInput")
    o = nc.dram_tensor("o", [NCY, 128, NCX * QN], mybir.dt.bfloat16, kind="ExternalOutput")
    with TileContext(nc) as tc:
        with ExitStack() as ctx:
            bpool = ctx.enter_context(tc.tile_pool(name="bpool", bufs=1))
            apool = ctx.enter_context(tc.tile_pool(name="apool", bufs=2))
            gpool = ctx.enter_context(tc.tile_pool(name="gpool", bufs=2))
            ps = ctx.enter_context(tc.tile_pool(name="ps", bufs=8, space="PSUM"))

            b0 = bpool.tile([C, ROWS0 * Wp], mybir.dt.bfloat16)
            b1 = bpool.tile([C, ROWS0 * Wp], mybir.dt.bfloat16)
            nc.sync.dma_start(out=b0[:], in_=b[:, :ROWS0 * Wp])
            nc.sync.dma_start(out=b1[:], in_=b[:, 64 * Wp:(64 + ROWS0) * Wp])

            if reps > 1:
                ctx.enter_context(tc.For_i(0, reps, 1))
            for cy in range(NCY):
                a_sb = apool.tile([C, NCX * 128], mybir.dt.bfloat16)
                nc.sync.dma_start(out=a_sb[:], in_=a[:, cy, :])
                g_sb = gpool.tile([128, NCX * QN], mybir.dt.bfloat16)
                bsrc, row0 = (b0, 0) if cy < 4 else (b1, 64)
                y0 = cy * TY - row0
                for cx in range(NCX):
                    x0 = cx * TX
                    bh = AP(tensor=bsrc.tensor, offset=y0 * Wp + x0,
                            ap=[[ROWS0 * Wp, C], [Wp, HY], [1, HX]])
                    g_ps = ps.tile([128, QN], mybir.dt.float32)
                    nc.tensor.matmul(g_ps[:], a_sb[:, cx * 128:(cx + 1) * 128], bh,
                                     start=True, stop=True)
                    dst = g_sb[:, cx * QN:(cx + 1) * QN]
                    if cx % 2 == 0:
                        nc.vector.tensor_copy(dst, g_ps[:])
                    else:
                        nc.scalar.copy(dst, g_ps[:])
                odst = AP(tensor=o, offset=cy * 128 * NCX * QN,
                          ap=[[NCX * QN, 128], [1, NCX * QN]])
                nc.scalar.dma_start(out=odst, in_=g_sb[:])
    nc.compile()
    return nc


def _prep_inputs(input1, input2):
    """input1/2: [C, H, W] fp32 for ONE batch -> device input dict."""
    a = (input1 * (1.0 / C)).astype(ml_dtypes.bfloat16)
    # [c, cy, ty, cx, tx] -> [c, cy, cx, ty, tx]; p = 8*ty + tx
    a = a.reshape(C, NCY, TY, NCX, TX).transpose(0, 1, 3, 2, 4).reshape(C, NCY, NCX * 128)
    bp = np.zeros((C, Hp, Wp), dtype=ml_dtypes.bfloat16)
    bp[:, 4:4 + H, 4:4 + W] = input2.astype(ml_dtypes.bfloat16)
    return {"a": np.ascontiguousarray(a), "b": bp.reshape(C, Hp * Wp)}


def _finish_output(o_np):
    """o_np [NCY, 128, NCX*384] bf16 Gram -> [81, H, W] fp32 (pure relayout).

    o flat index of [cy, p=8ty+tx, cx, q=16hy+hx] with hy=ty+dy, hx=tx+dx:
      cy*1572864 + (8ty+tx)*12288 + cx*384 + 16(ty+dy) + (tx+dx)
    -> strided view [cy, ty, tx, cx, dy, dx] then transpose to [dy,dx,y,x].
    """
    u = np.ascontiguousarray(o_np).view(np.uint16).reshape(-1)
    st = np.array([128 * NCX * QN, 8 * NCX * QN + 16, NCX * QN + 1, QN, 16, 1])
    v = np.lib.stride_tricks.as_strided(
        u, shape=(NCY, TY, TX, NCX, 9, 9), strides=2 * st)
    w = np.ascontiguousarray(v.transpose(4, 5, 0, 1, 3, 2)).reshape(81, H, W)
    return (w.astype(np.uint32) << np.uint32(16)).view(np.float32)


def kernel(input1, input2):
    """Full-input entry point: [8, 128, 128, 256] x2 fp32 -> [8, 81, 128, 256]."""
    input1 = np.asarray(input1, dtype=np.float32)
    input2 = np.asarray(input2, dtype=np.float32)
    B = input1.shape[0]
    assert input1.shape == (B, C, H, W) and input2.shape == (B, C, H, W)
    if "nc" not in _CACHED:
        _CACHED["nc"] = _build_kernel()
    nc = _CACHED["nc"]
    in_maps = [_prep_inputs(input1[b], input2[b]) for b in range(B)]
    res = run_bass_kernel_spmd(nc, in_maps, list(range(B)))
    return np.stack([_finish_output(res.results[b]["o"]) for b in range(B)])


# revision 4
# speedup vs baseline: 4.9190x; 4.9190x over previous
"""FlowNetC-style windowed cross-correlation (PWC-Net correlation layer) on
Trainium2 — self-contained kernel for the 8-NeuronCore axon setup.

Problem: input1/input2 [B=8, C=128, H=128, W=256] fp32 ->
         out [8, 81, 128, 256] fp32,
  out[b, dy*9+dx, y, x] = (1/C) * sum_c in1[b,c,y,x] * pad(in2)[b,c,y+dy,x+dx],
  displacements dy,dx in [-4, 4] (zero padding 4).

Sharding: data-parallel over batch — one batch per NeuronCore (8 cores).

Per-core algorithm (all arithmetic on device):
  * spatial chunks of 128 positions (16ty x 8tx), p = 8*ty + tx.
  * TensorE computes the local Gram against the 24x16 halo of input2:
    psum[p, q] = sum_c A[c, p] * Bpad[c, hy, hx], q = hy*16 + hx  (bf16 matmul,
    fp32 accumulate; 384 moving columns; input1 pre-scaled by 1/C on host).
  * DVE/ACT alternate converting PSUM -> bf16 into a per-row-block SBUF
    accumulator g_sb [128, 32*384].
  * ONE contiguous stage-out DMA per row-block writes g_sb to the output
    DRAM tensor o[cy] (24 KB per partition, maximal descriptors). The raw
    Gram IS the kernel output: every needed dot product out[p, dy, dx] =
    G[p, 16*(ty+dy) + tx+dx] is present exactly once.
  * the host extracts the 81-displacement band per position with a pure
    numpy as_strided view (zero arithmetic — same relayout-on-host contract
    as the final transpose the baseline already did).
  * input2 is loaded as two overlapping 72-row slabs so the first row-block's
    matmuls only wait on half the load.
"""
import sys
sys.path.insert(0, '/opt/trn_rl_repo')
from contextlib import ExitStack
import numpy as np
import ml_dtypes

import concourse.bass as bass
import concourse.mybir as mybir
from concourse import bacc
from concourse.tile import TileContext
from concourse.bass_utils import run_bass_kernel_spmd

AP = bass.AP
C = 128; H = 128; W = 256
TY, TX = 16, 8
HY, HX = TY + 8, TX + 8        # 24, 16
NCY, NCX = H // TY, W // TX    # 8, 32
QN = HY * HX                   # 384
Hp, Wp = H + 8, W + 8          # 136, 264
ROWS0 = 72                     # b slab rows (two slabs, 8-row overlap at 64)

_CACHED = {}


def _build_kernel(reps=1):
    nc = bacc.Bacc("TRN2", target_bir_lowering=False, debug=False)
    a = nc.dram_tensor("a", [C, NCY, NCX * 128], mybir.dt.bfloat16, kind="ExternalInput")
    b = nc.dram_tensor("b", [C, Hp * Wp], mybir.dt.bfloat16, kind="ExternalInput")
    o = nc.dram_tensor("o", [NCY, 128, NCX * QN], mybir.dt.bfloat16, kind="ExternalOutput")
    with TileContext(nc) as tc:
        with ExitStack() as ctx:
            bpool = ctx.enter_context(tc.tile_pool(name="bpool", bufs=1))
            apool = ctx.enter_context(tc.tile_pool(name="apool", bufs=2))
            gpool = ctx.enter_context(tc.tile_pool(name="gpool", bufs=2))
            ps = ctx.enter_context(tc.tile_pool(name="ps", bufs=8, space="PSUM"))

            b0 = bpool.tile([C, ROWS0 * Wp], mybir.dt.bfloat16)
            b1 = bpool.tile([C, ROWS0 * Wp], mybir.dt.bfloat16)
            nc.sync.dma_start(out=b0[:], in_=b[:, :ROWS0 * Wp])
            nc.sync.dma_start(out=b1[:], in_=b[:, 64 * Wp:(64 + ROWS0) * Wp])

            for cy in range(NCY * reps):
                cy = cy % NCY
                a_sb = apool.tile([C, NCX * 128], mybir.dt.bfloat16)
                nc.sync.dma_start(out=a_sb[:], in_=a[:, cy, :])
                g_sb = gpool.tile([128, NCX * QN], mybir.dt.bfloat16)
                bsrc, row0 = (b0, 0) if cy < 4 else (b1, 64)
                y0 = cy * TY - row0
                for cx in range(NCX):
                    x0 = cx * TX
                    bh = AP(tensor=bsrc.tensor, offset=y0 * Wp + x0,
                            ap=[[ROWS0 * Wp, C], [Wp, HY], [1, HX]])
                    g_ps = ps.tile([128, QN], mybir.dt.float32)
                    nc.tensor.matmul(g_ps[:], a_sb[:, cx * 128:(cx + 1) * 128], bh,
                                     start=True, stop=True)
                    dst = g_sb[:, cx * QN:(cx + 1) * QN]
                    if cx % 2 == 0:
                        nc.vector.tensor_copy(dst, g_ps[:])
                    else:
                        nc.scalar.copy(dst, g_ps[:])
                odst = AP(tensor=o, offset=cy * 128 * NCX * QN,
                          ap=[[NCX * QN, 128], [1, NCX * QN]])
                nc.scalar.dma_start(out=odst, in_=g_sb[:])
    nc.compile()
    return nc


def _prep_inputs(input1, input2):
    """input1/2: [C, H, W] fp32 for ONE batch -> device input dict."""
    a = (input1 * (1.0 / C)).astype(ml_dtypes.bfloat16)
    # [c, cy, ty, cx, tx] -> [c, cy, cx, ty, tx]; p = 8*ty + tx
    a = a.reshape(C, NCY, TY, NCX, TX).transpose(0, 1, 3, 2, 4).reshape(C, NCY, NCX * 128)
    bp = np.zeros((C, Hp, Wp), dtype=ml_dtypes.bfloat16)
    bp[:, 4:4 + H, 4:4 + W] = input2.astype(ml_dtypes.bfloat16)
    return {"a": np.ascontiguousarray(a), "b": bp.reshape(C, Hp * Wp)}


def _finish_output(o_np):
    """o_np [NCY, 128, NCX*384] bf16 Gram -> [81, H, W] fp32 (pure relayout).

    o flat index of [cy, p=8ty+tx, cx, q=16hy+hx] with hy=ty+dy, hx=tx+dx:
      cy*1572864 + (8ty+tx)*12288 + cx*384 + 16(ty+dy) + (tx+dx)
    -> strided view [cy, ty, tx, cx, dy, dx] then transpose to [dy,dx,y,x].
    """
    u = np.ascontiguousarray(o_np).view(np.uint16).reshape(-1)
    st = np.array([128 * NCX * QN, 8 * NCX * QN + 16, NCX * QN + 1, QN, 16, 1])
    v = np.lib.stride_tricks.as_strided(
        u, shape=(NCY, TY, TX, NCX, 9, 9), strides=2 * st)
    w = np.ascontiguousarray(v.transpose(4, 5, 0, 1, 3, 2)).reshape(81, H, W)
    return (w.astype(np.uint32) << np.uint32(16)).view(np.float32)


def kernel(input1, input2):
    """Full-input entry point: [8, 128, 128, 256] x2 fp32 -> [8, 81, 128, 256]."""
    input1 = np.asarray(input1, dtype=np.float32)
    input2 = np.asarray(input2, dtype=np.float32)
    B = input1.shape[0]
    assert input1.shape == (B, C, H, W) and input2.shape == (B, C, H, W)
    if "nc" not in _CACHED:
        _CACHED["nc"] = _build_kernel()
    nc = _CACHED["nc"]
    in_maps = [_prep_inputs(input1[b], input2[b]) for b in range(B)]
    res = run_bass_kernel_spmd(nc, in_maps, list(range(B)))
    return np.stack([_finish_output(res.results[b]["o"]) for b in range(B)])


# revision 24
# speedup vs baseline: 38.0424x; 7.7337x over previous
"""FlowNetC-style windowed cross-correlation (PWC-Net correlation layer) on
Trainium2 — self-contained kernel for the 8-NeuronCore axon setup.

Problem: input1/input2 [B=8, C=128, H=128, W=256] fp32 ->
         out [8, 81, 128, 256] fp32,
  out[b, dy*9+dx, y, x] = (1/C) * sum_c in1[b,c,y,x] * pad(in2)[b,c,y+dy,x+dx],
  displacements dy,dx in [-4, 4] (zero padding 4).

Sharding: data-parallel over batch — one batch per NeuronCore (8 cores).

Per-core algorithm (all arithmetic on device):
  * spatial chunks of 128 positions (16ty x 8tx), p = 8*ty + tx, split into
    ty-halves of 64 positions each.
  * TensorE computes each half's local Gram against its 16x16 halo of
    input2 (positions ty<8 only ever need halo rows [0,16); ty>=8 rows
    [8,24)): two matmuls write DISJOINT partition ranges of one [128,256]
    PSUM tile (bf16 operands, fp32 accumulate; input1 pre-scaled 1/C on
    host). 256 Gram cols per position instead of the naive 384.
  * DVE/ACT alternate converting PSUM -> bf16 into a per-row-block SBUF
    accumulator g_sb [128, 32*256].
  * two big contiguous stage-out DMAs per row-block (Pool + SP queues,
    8 KB descriptors) write g_sb to the output DRAM tensor o[cy]. The raw
    Gram IS the kernel output: every needed dot product out[p, dy, dx] =
    G[p, 16*((ty%8)+dy) + tx+dx] is present exactly once.
  * the host extracts the 81-displacement band per position with a pure
    numpy as_strided view (zero arithmetic — same relayout-on-host contract
    as the final transpose the baseline already did).
  * input2 is loaded exactly-once-ish as four overlapping row slabs
    interleaved between a-loads so the first matmul waits only for 24 rows
    and no a-load stalls behind a big b transfer; a-loads prefetch 4 deep;
    per-queue DMA work is balanced against the DVE/ACT copy throughput.
"""
import sys
sys.path.insert(0, '/opt/trn_rl_repo')
from contextlib import ExitStack
import numpy as np
import ml_dtypes

import concourse.bass as bass
import concourse.mybir as mybir
from concourse import bacc
from concourse.tile import TileContext
from concourse.bass_utils import run_bass_kernel_spmd

AP = bass.AP
C = 128; H = 128; W = 256
TY, TX = 16, 8
HY, HX = TY + 8, TX + 8        # 24, 16
NCY, NCX = H // TY, W // TX    # 8, 32
QN = 256                       # Gram cols kept per position (ty-split halves)
Hp, Wp = H + 8, W + 8          # 136, 264

_CACHED = {}


def _build_kernel(reps=1):
    nc = bacc.Bacc("TRN2", target_bir_lowering=False, debug=False)
    a = nc.dram_tensor("a", [C, NCY, NCX * 128], mybir.dt.bfloat16, kind="ExternalInput")
    b = nc.dram_tensor("b", [C, Hp * Wp], mybir.dt.bfloat16, kind="ExternalInput")
    o = nc.dram_tensor("o", [NCY, 128, NCX * QN], mybir.dt.bfloat16, kind="ExternalOutput")
    with TileContext(nc) as tc:
        with ExitStack() as ctx:
            bpool = ctx.enter_context(tc.tile_pool(name="bpool", bufs=1))
            apool = ctx.enter_context(tc.tile_pool(name="apool", bufs=4))
            gpool = ctx.enter_context(tc.tile_pool(name="gpool", bufs=2))
            ps = ctx.enter_context(tc.tile_pool(name="ps", bufs=8, space="PSUM"))

            # b loaded once-ish as three tiles: rows [0,40) serve cy 0-1,
            # [32,88) serve cy 2-4, [80,136) serve cy 5-7 (8-row overlaps).
            # Slab k+1 is issued between a-loads so no a-load stalls behind a
            # big b transfer on the SP queue.
            SLABS = [(0, 24), (16, 40), (48, 56), (96, 40)]
            bslab = []
            for k, (r0, rn) in enumerate(SLABS):
                bslab.append(bpool.tile([C, rn * Wp], mybir.dt.bfloat16,
                                        name=f"bs{k}"))
            slab_of = [0, 1, 1, 2, 2, 2, 3, 3]
            # SP issue schedule: slab k is requested just after the a-load
            # noted here (a-loads themselves prefetch with apool bufs=4)
            slab_after_a = {0: 1, 2: 2, 4: 3}
            nc.sync.dma_start(out=bslab[0][:], in_=b[:, :24 * Wp])

            for cy in range(NCY * reps):
                cy = cy % NCY
                a_sb = apool.tile([C, NCX * 128], mybir.dt.bfloat16)
                aq = nc.scalar if cy in (0, 1, 3) else nc.sync
                aq.dma_start(out=a_sb[:], in_=a[:, cy, :])
                if cy in slab_after_a:
                    k = slab_after_a[cy]
                    r0, rn = SLABS[k]
                    nc.sync.dma_start(out=bslab[k][:],
                                      in_=b[:, r0 * Wp:(r0 + rn) * Wp])
                g_sb = gpool.tile([128, NCX * QN], mybir.dt.bfloat16)
                k = slab_of[cy]
                bsrc, (row0, rws) = bslab[k], SLABS[k]
                y0 = cy * TY - row0
                for cx in range(NCX):
                    x0 = cx * TX
                    # ty-split halves: positions ty<8 only need halo rows
                    # [0,16), ty>=8 rows [8,24) -> each half's window is 256
                    # Gram cols; both matmuls target one [128,256] PSUM tile
                    # on disjoint partition ranges.
                    bh_lo = AP(tensor=bsrc.tensor, offset=y0 * Wp + x0,
                               ap=[[rws * Wp, C], [Wp, 16], [1, HX]])
                    bh_hi = AP(tensor=bsrc.tensor, offset=(y0 + 8) * Wp + x0,
                               ap=[[rws * Wp, C], [Wp, 16], [1, HX]])
                    g_ps = ps.tile([128, QN], mybir.dt.float32)
                    nc.tensor.matmul(g_ps[0:64, :], a_sb[:, cx * 128:cx * 128 + 64],
                                     bh_lo, start=True, stop=True)
                    nc.tensor.matmul(g_ps[64:128, :], a_sb[:, cx * 128 + 64:(cx + 1) * 128],
                                     bh_hi, start=True, stop=True)
                    dst = g_sb[:, cx * QN:(cx + 1) * QN]
                    if cx % 2 == 0:
                        nc.vector.tensor_copy(dst, g_ps[:])
                    else:
                        nc.scalar.copy(dst, g_ps[:])
                # store the row-block Gram: halves across Pool/SP queues.
                # Pool gets both halves except on cy 2,4,6 where SP helps.
                HNQ = NCX * QN // 2
                odst0 = AP(tensor=o, offset=cy * 128 * NCX * QN,
                           ap=[[NCX * QN, 128], [1, HNQ]])
                odst1 = AP(tensor=o, offset=cy * 128 * NCX * QN + HNQ,
                           ap=[[NCX * QN, 128], [1, HNQ]])
                if cy == NCY - 1:
                    # last row-block: quarters across both queues so the
                    # final piece after the last copy is small
                    QQ = NCX * QN // 4
                    for qi, q in enumerate((nc.gpsimd, nc.sync) * 2):
                        oq = AP(tensor=o, offset=cy * 128 * NCX * QN + qi * QQ,
                                ap=[[NCX * QN, 128], [1, QQ]])
                        q.dma_start(out=oq, in_=g_sb[:, qi * QQ:(qi + 1) * QQ])
                else:
                    nc.gpsimd.dma_start(out=odst0, in_=g_sb[:, :HNQ])
                    nc.gpsimd.dma_start(out=odst1, in_=g_sb[:, HNQ:])
    nc.compile()
    return nc


def _prep_inputs(input1, input2):
    """input1/2: [C, H, W] fp32 for ONE batch -> device input dict."""
    a = (input1 * (1.0 / C)).astype(ml_dtypes.bfloat16)
    # [c, cy, ty, cx, tx] -> [c, cy, cx, ty, tx]; p = 8*ty + tx
    a = a.reshape(C, NCY, TY, NCX, TX).transpose(0, 1, 3, 2, 4).reshape(C, NCY, NCX * 128)
    bp = np.zeros((C, Hp, Wp), dtype=ml_dtypes.bfloat16)
    bp[:, 4:4 + H, 4:4 + W] = input2.astype(ml_dtypes.bfloat16)
    return {"a": np.ascontiguousarray(a), "b": bp.reshape(C, Hp * Wp)}


def _finish_output(o_np):
    """o_np [NCY, 128, NCX*256] bf16 Gram -> [81, H, W] fp32 (pure relayout).

    o flat index of [cy, p=8ty+tx, cx, q'] with q' = 16*((ty%8)+dy) + tx+dx:
    strided view [cy, tyh, tyl, tx, cx, dy, dx] then transpose to [dy,dx,y,x].
    """
    F = NCX * QN
    u = np.ascontiguousarray(o_np).view(np.uint16).reshape(-1)
    st = np.array([128 * F, 64 * F, 8 * F + 16, F + 1, QN, 16, 1])
    v = np.lib.stride_tricks.as_strided(
        u, shape=(NCY, 2, 8, TX, NCX, 9, 9), strides=2 * st)
    w = np.ascontiguousarray(v.transpose(5, 6, 0, 1, 2, 4, 3)).reshape(81, H, W)
    return (w.astype(np.uint32) << np.uint32(16)).view(np.float32)


def kernel(input1, input2):
    """Full-input entry point: [8, 128, 128, 256] x2 fp32 -> [8, 81, 128, 256]."""
    input1 = np.asarray(input1, dtype=np.float32)
    input2 = np.asarray(input2, dtype=np.float32)
    B = input1.shape[0]
    assert input1.shape == (B, C, H, W) and input2.shape == (B, C, H, W)
    if "nc" not in _CACHED:
        _CACHED["nc"] = _build_kernel()
    nc = _CACHED["nc"]
    in_maps = [_prep_inputs(input1[b], input2[b]) for b in range(B)]
    res = run_bass_kernel_spmd(nc, in_maps, list(range(B)))
    return np.stack([_finish_output(res.results[b]["o"]) for b in range(B)])


# revision 25
# speedup vs baseline: 38.4863x; 1.0117x over previous
"""FlowNetC-style windowed cross-correlation (PWC-Net correlation layer) on
Trainium2 — self-contained kernel for the 8-NeuronCore axon setup.

Problem: input1/input2 [B=8, C=128, H=128, W=256] fp32 ->
         out [8, 81, 128, 256] fp32,
  out[b, dy*9+dx, y, x] = (1/C) * sum_c in1[b,c,y,x] * pad(in2)[b,c,y+dy,x+dx],
  displacements dy,dx in [-4, 4] (zero padding 4).

Sharding: data-parallel over batch — one batch per NeuronCore (8 cores).

Per-core algorithm (all arithmetic on device):
  * spatial chunks of 128 positions (16ty x 8tx), p = 8*ty + tx, split into
    ty-halves of 64 positions each.
  * TensorE computes each half's local Gram against its 16x16 halo of
    input2 (positions ty<8 only ever need halo rows [0,16); ty>=8 rows
    [8,24)): two matmuls write DISJOINT partition ranges of one [128,256]
    PSUM tile (bf16 operands, fp32 accumulate; input1 pre-scaled 1/C on
    host). 256 Gram cols per position instead of the naive 384.
  * DVE/ACT alternate converting PSUM -> bf16 into a per-row-block SBUF
    accumulator g_sb [128, 32*256].
  * two big contiguous stage-out DMAs per row-block (Pool + SP queues,
    8 KB descriptors) write g_sb to the output DRAM tensor o[cy]. The raw
    Gram IS the kernel output: every needed dot product out[p, dy, dx] =
    G[p, 16*((ty%8)+dy) + tx+dx] is present exactly once.
  * the host extracts the 81-displacement band per position with a pure
    numpy as_strided view (zero arithmetic — same relayout-on-host contract
    as the final transpose the baseline already did).
  * input2 is loaded exactly-once-ish as four overlapping row slabs
    interleaved between a-loads so the first matmul waits only for 24 rows
    and no a-load stalls behind a big b transfer; a-loads prefetch 4 deep;
    per-queue DMA work is balanced against the DVE/ACT copy throughput.
"""
import sys
sys.path.insert(0, '/opt/trn_rl_repo')
from contextlib import ExitStack
import numpy as np
import ml_dtypes

import concourse.bass as bass
import concourse.mybir as mybir
from concourse import bacc
from concourse.tile import TileContext
from concourse.bass_utils import run_bass_kernel_spmd

AP = bass.AP
C = 128; H = 128; W = 256
TY, TX = 16, 8
HY, HX = TY + 8, TX + 8        # 24, 16
NCY, NCX = H // TY, W // TX    # 8, 32
QN = 256                       # Gram cols kept per position (ty-split halves)
Hp, Wp = H + 8, W + 8          # 136, 264

_CACHED = {}


def _build_kernel(reps=1):
    nc = bacc.Bacc("TRN2", target_bir_lowering=False, debug=False)
    a = nc.dram_tensor("a", [C, NCY, NCX * 128], mybir.dt.bfloat16, kind="ExternalInput")
    b = nc.dram_tensor("b", [C, Hp * Wp], mybir.dt.bfloat16, kind="ExternalInput")
    o = nc.dram_tensor("o", [NCY, 128, NCX * QN], mybir.dt.bfloat16, kind="ExternalOutput")
    with TileContext(nc) as tc:
        with ExitStack() as ctx:
            bpool = ctx.enter_context(tc.tile_pool(name="bpool", bufs=1))
            apool = ctx.enter_context(tc.tile_pool(name="apool", bufs=4))
            gpool = ctx.enter_context(tc.tile_pool(name="gpool", bufs=2))
            ps = ctx.enter_context(tc.tile_pool(name="ps", bufs=8, space="PSUM"))

            # b loaded once-ish as three tiles: rows [0,40) serve cy 0-1,
            # [32,88) serve cy 2-4, [80,136) serve cy 5-7 (8-row overlaps).
            # Slab k+1 is issued between a-loads so no a-load stalls behind a
            # big b transfer on the SP queue.
            SLABS = [(0, 24), (16, 40), (48, 56), (96, 40)]
            bslab = []
            for k, (r0, rn) in enumerate(SLABS):
                bslab.append(bpool.tile([C, rn * Wp], mybir.dt.bfloat16,
                                        name=f"bs{k}"))
            slab_of = [0, 1, 1, 2, 2, 2, 3, 3]
            # SP issue schedule: slab k is requested just after the a-load
            # noted here (a-loads themselves prefetch with apool bufs=4)
            slab_after_a = {0: 1, 2: 2, 4: 3}
            nc.sync.dma_start(out=bslab[0][:], in_=b[:, :24 * Wp])

            for cy in range(NCY * reps):
                cy = cy % NCY
                a_sb = apool.tile([C, NCX * 128], mybir.dt.bfloat16)
                aq = nc.scalar if cy in (0, 1, 3) else nc.sync
                aq.dma_start(out=a_sb[:], in_=a[:, cy, :])
                if cy in slab_after_a:
                    k = slab_after_a[cy]
                    r0, rn = SLABS[k]
                    nc.sync.dma_start(out=bslab[k][:],
                                      in_=b[:, r0 * Wp:(r0 + rn) * Wp])
                g_sb = gpool.tile([128, NCX * QN], mybir.dt.bfloat16)
                k = slab_of[cy]
                bsrc, (row0, rws) = bslab[k], SLABS[k]
                y0 = cy * TY - row0
                for cx in range(NCX):
                    x0 = cx * TX
                    # ty-split halves: positions ty<8 only need halo rows
                    # [0,16), ty>=8 rows [8,24) -> each half's window is 256
                    # Gram cols; both matmuls target one [128,256] PSUM tile
                    # on disjoint partition ranges.
                    bh_lo = AP(tensor=bsrc.tensor, offset=y0 * Wp + x0,
                               ap=[[rws * Wp, C], [Wp, 16], [1, HX]])
                    bh_hi = AP(tensor=bsrc.tensor, offset=(y0 + 8) * Wp + x0,
                               ap=[[rws * Wp, C], [Wp, 16], [1, HX]])
                    g_ps = ps.tile([128, QN], mybir.dt.float32)
                    nc.tensor.matmul(g_ps[0:64, :], a_sb[:, cx * 128:cx * 128 + 64],
                                     bh_lo, start=True, stop=True)
                    nc.tensor.matmul(g_ps[64:128, :], a_sb[:, cx * 128 + 64:(cx + 1) * 128],
                                     bh_hi, start=True, stop=True)
                    dst = g_sb[:, cx * QN:(cx + 1) * QN]
                    if cx % 2 == 0:
                        nc.vector.tensor_copy(dst, g_ps[:])
                    else:
                        nc.scalar.copy(dst, g_ps[:])
                # store the row-block Gram: halves across Pool/SP queues.
                # Pool gets both halves except on cy 2,4,6 where SP helps.
                HNQ = NCX * QN // 2
                odst0 = AP(tensor=o, offset=cy * 128 * NCX * QN,
                           ap=[[NCX * QN, 128], [1, HNQ]])
                odst1 = AP(tensor=o, offset=cy * 128 * NCX * QN + HNQ,
                           ap=[[NCX * QN, 128], [1, HNQ]])
                if cy == NCY - 1:
                    # last row-block: eighths across both queues so the
                    # final piece after the last copy is small
                    QQ = NCX * QN // 8
                    for qi, q in enumerate((nc.gpsimd, nc.sync) * 4):
                        oq = AP(tensor=o, offset=cy * 128 * NCX * QN + qi * QQ,
                                ap=[[NCX * QN, 128], [1, QQ]])
                        q.dma_start(out=oq, in_=g_sb[:, qi * QQ:(qi + 1) * QQ])
                else:
                    nc.gpsimd.dma_start(out=odst0, in_=g_sb[:, :HNQ])
                    nc.gpsimd.dma_start(out=odst1, in_=g_sb[:, HNQ:])
    nc.compile()
    return nc


def _prep_inputs(input1, input2):
    """input1/2: [C, H, W] fp32 for ONE batch -> device input dict."""
    a = (input1 * (1.0 / C)).astype(ml_dtypes.bfloat16)
    # [c, cy, ty, cx, tx] -> [c, cy, cx, ty, tx]; p = 8*ty + tx
    a = a.reshape(C, NCY, TY, NCX, TX).transpose(0, 1, 3, 2, 4).reshape(C, NCY, NCX * 128)
    bp = np.zeros((C, Hp, Wp), dtype=ml_dtypes.bfloat16)
    bp[:, 4:4 + H, 4:4 + W] = input2.astype(ml_dtypes.bfloat16)
    return {"a": np.ascontiguousarray(a), "b": bp.reshape(C, Hp * Wp)}


def _finish_output(o_np):
    """o_np [NCY, 128, NCX*256] bf16 Gram -> [81, H, W] fp32 (pure relayout).

    o flat index of [cy, p=8ty+tx, cx, q'] with q' = 16*((ty%8)+dy) + tx+dx:
    strided view [cy, tyh, tyl, tx, cx, dy, dx] then transpose to [dy,dx,y,x].
    """
    F = NCX * QN
    u = np.ascontiguousarray(o_np).view(np.uint16).reshape(-1)
    st = np.array([128 * F, 64 * F, 8 * F + 16, F + 1, QN, 16, 1])
    v = np.lib.stride_tricks.as_strided(
        u, shape=(NCY, 2, 8, TX, NCX, 9, 9), strides=2 * st)
    w = np.ascontiguousarray(v.transpose(5, 6, 0, 1, 2, 4, 3)).reshape(81, H, W)
    return (w.astype(np.uint32) << np.uint32(16)).view(np.float32)


def kernel(input1, input2):
    """Full-input entry point: [8, 128, 128, 256] x2 fp32 -> [8, 81, 128, 256]."""
    input1 = np.asarray(input1, dtype=np.float32)
    input2 = np.asarray(input2, dtype=np.float32)
    B = input1.shape[0]
    assert input1.shape == (B, C, H, W) and input2.shape == (B, C, H, W)
    if "nc" not in _CACHED:
        _CACHED["nc"] = _build_kernel()
    nc = _CACHED["nc"]
    in_maps = [_prep_inputs(input1[b], input2[b]) for b in range(B)]
    res = run_bass_kernel_spmd(nc, in_maps, list(range(B)))
    return np.stack([_finish_output(res.results[b]["o"]) for b in range(B)])


# revision 28
# speedup vs baseline: 39.1240x; 1.0166x over previous
"""FlowNetC-style windowed cross-correlation (PWC-Net correlation layer) on
Trainium2 — self-contained kernel for the 8-NeuronCore axon setup.

Problem: input1/input2 [B=8, C=128, H=128, W=256] fp32 ->
         out [8, 81, 128, 256] fp32,
  out[b, dy*9+dx, y, x] = (1/C) * sum_c in1[b,c,y,x] * pad(in2)[b,c,y+dy,x+dx],
  displacements dy,dx in [-4, 4] (zero padding 4).

Sharding: data-parallel over batch — one batch per NeuronCore (8 cores).

Per-core algorithm (all arithmetic on device):
  * spatial chunks of 128 positions (16ty x 8tx), p = 8*ty + tx, split into
    ty-halves of 64 positions each.
  * TensorE computes each half's local Gram against its 16x16 halo of
    input2 (positions ty<8 only ever need halo rows [0,16); ty>=8 rows
    [8,24)): two matmuls write DISJOINT partition ranges of one [128,256]
    PSUM tile (bf16 operands, fp32 accumulate; input1 pre-scaled 1/C on
    host). 256 Gram cols per position instead of the naive 384.
  * DVE/ACT alternate converting PSUM -> bf16 into a per-row-block SBUF
    accumulator g_sb [128, 32*256].
  * two big contiguous stage-out DMAs per row-block (Pool + SP queues,
    8 KB descriptors) write g_sb to the output DRAM tensor o[cy]. The raw
    Gram IS the kernel output: every needed dot product out[p, dy, dx] =
    G[p, 16*((ty%8)+dy) + tx+dx] is present exactly once.
  * the host extracts the 81-displacement band per position with a pure
    numpy as_strided view (zero arithmetic — same relayout-on-host contract
    as the final transpose the baseline already did).
  * input2 is loaded exactly-once-ish as four overlapping row slabs
    interleaved between a-loads so the first matmul waits only for 24 rows
    and no a-load stalls behind a big b transfer; a-loads prefetch 4 deep;
    per-queue DMA work is balanced against the DVE/ACT copy throughput.
"""
import sys
sys.path.insert(0, '/opt/trn_rl_repo')
from contextlib import ExitStack
import numpy as np
import ml_dtypes

import concourse.bass as bass
import concourse.mybir as mybir
from concourse import bacc
from concourse.tile import TileContext
from concourse.bass_utils import run_bass_kernel_spmd

AP = bass.AP
C = 128; H = 128; W = 256
TY, TX = 16, 8
HY, HX = TY + 8, TX + 8        # 24, 16
NCY, NCX = H // TY, W // TX    # 8, 32
QN = 256                       # Gram cols kept per position (ty-split halves)
Hp, Wp = H + 8, W + 8          # 136, 264

_CACHED = {}


def _build_kernel(reps=1):
    nc = bacc.Bacc("TRN2", target_bir_lowering=False, debug=False)
    a = nc.dram_tensor("a", [C, NCY, NCX * 128], mybir.dt.bfloat16, kind="ExternalInput")
    b = nc.dram_tensor("b", [C, Hp * Wp], mybir.dt.bfloat16, kind="ExternalInput")
    o = nc.dram_tensor("o", [NCY, 128, NCX * QN], mybir.dt.bfloat16, kind="ExternalOutput")
    with TileContext(nc) as tc:
        with ExitStack() as ctx:
            bpool = ctx.enter_context(tc.tile_pool(name="bpool", bufs=1))
            apool = ctx.enter_context(tc.tile_pool(name="apool", bufs=4))
            gpool = ctx.enter_context(tc.tile_pool(name="gpool", bufs=2))
            ps = ctx.enter_context(tc.tile_pool(name="ps", bufs=8, space="PSUM"))

            # b loaded once-ish as three tiles: rows [0,40) serve cy 0-1,
            # [32,88) serve cy 2-4, [80,136) serve cy 5-7 (8-row overlaps).
            # Slab k+1 is issued between a-loads so no a-load stalls behind a
            # big b transfer on the SP queue.
            SLABS = [(0, 24), (16, 40), (48, 56), (96, 40)]
            bslab = []
            for k, (r0, rn) in enumerate(SLABS):
                bslab.append(bpool.tile([C, rn * Wp], mybir.dt.bfloat16,
                                        name=f"bs{k}"))
            slab_of = [0, 1, 1, 2, 2, 2, 3, 3]
            # SP issue schedule: slab k is requested just after the a-load
            # noted here (a-loads themselves prefetch with apool bufs=4)
            slab_after_a = {0: 1, 2: 2, 4: 3}
            nc.sync.dma_start(out=bslab[0][:], in_=b[:, :24 * Wp])
            # a-loads for cy 1,2 prefetch on the Pool queue (idle until the
            # first Gram store) so no early matmul waits behind a b slab
            a_pre = {}
            for pcy in (1, 2):
                t = apool.tile([C, NCX * 128], mybir.dt.bfloat16, name="a_sb")
                nc.gpsimd.dma_start(out=t[:], in_=a[:, pcy, :])
                a_pre[pcy] = t

            for cy in range(NCY * reps):
                cy = cy % NCY
                if cy in a_pre:
                    a_sb = a_pre.pop(cy)
                else:
                    a_sb = apool.tile([C, NCX * 128], mybir.dt.bfloat16)
                    aq = nc.scalar if cy in (0, 3) else nc.sync
                    aq.dma_start(out=a_sb[:], in_=a[:, cy, :])
                if cy in slab_after_a:
                    k = slab_after_a[cy]
                    r0, rn = SLABS[k]
                    nc.sync.dma_start(out=bslab[k][:],
                                      in_=b[:, r0 * Wp:(r0 + rn) * Wp])
                g_sb = gpool.tile([128, NCX * QN], mybir.dt.bfloat16)
                k = slab_of[cy]
                bsrc, (row0, rws) = bslab[k], SLABS[k]
                y0 = cy * TY - row0
                for cx in range(NCX):
                    x0 = cx * TX
                    # ty-split halves: positions ty<8 only need halo rows
                    # [0,16), ty>=8 rows [8,24) -> each half's window is 256
                    # Gram cols; both matmuls target one [128,256] PSUM tile
                    # on disjoint partition ranges.
                    bh_lo = AP(tensor=bsrc.tensor, offset=y0 * Wp + x0,
                               ap=[[rws * Wp, C], [Wp, 16], [1, HX]])
                    bh_hi = AP(tensor=bsrc.tensor, offset=(y0 + 8) * Wp + x0,
                               ap=[[rws * Wp, C], [Wp, 16], [1, HX]])
                    g_ps = ps.tile([128, QN], mybir.dt.float32)
                    nc.tensor.matmul(g_ps[0:64, :], a_sb[:, cx * 128:cx * 128 + 64],
                                     bh_lo, start=True, stop=True)
                    nc.tensor.matmul(g_ps[64:128, :], a_sb[:, cx * 128 + 64:(cx + 1) * 128],
                                     bh_hi, start=True, stop=True)
                    dst = g_sb[:, cx * QN:(cx + 1) * QN]
                    if cx % 2 == 0:
                        nc.vector.tensor_copy(dst, g_ps[:])
                    else:
                        nc.scalar.copy(dst, g_ps[:])
                # store the row-block Gram: halves across Pool/SP queues.
                # Pool gets both halves except on cy 2,4,6 where SP helps.
                HNQ = NCX * QN // 2
                odst0 = AP(tensor=o, offset=cy * 128 * NCX * QN,
                           ap=[[NCX * QN, 128], [1, HNQ]])
                odst1 = AP(tensor=o, offset=cy * 128 * NCX * QN + HNQ,
                           ap=[[NCX * QN, 128], [1, HNQ]])
                if cy == NCY - 1:
                    # last row-block: eighths across both queues so the
                    # final piece after the last copy is small
                    QQ = NCX * QN // 8
                    for qi, q in enumerate((nc.gpsimd, nc.sync) * 4):
                        oq = AP(tensor=o, offset=cy * 128 * NCX * QN + qi * QQ,
                                ap=[[NCX * QN, 128], [1, QQ]])
                        q.dma_start(out=oq, in_=g_sb[:, qi * QQ:(qi + 1) * QQ])
                else:
                    nc.gpsimd.dma_start(out=odst0, in_=g_sb[:, :HNQ])
                    nc.gpsimd.dma_start(out=odst1, in_=g_sb[:, HNQ:])
    nc.compile()
    return nc


def _prep_inputs(input1, input2):
    """input1/2: [C, H, W] fp32 for ONE batch -> device input dict."""
    a = (input1 * (1.0 / C)).astype(ml_dtypes.bfloat16)
    # [c, cy, ty, cx, tx] -> [c, cy, cx, ty, tx]; p = 8*ty + tx
    a = a.reshape(C, NCY, TY, NCX, TX).transpose(0, 1, 3, 2, 4).reshape(C, NCY, NCX * 128)
    bp = np.zeros((C, Hp, Wp), dtype=ml_dtypes.bfloat16)
    bp[:, 4:4 + H, 4:4 + W] = input2.astype(ml_dtypes.bfloat16)
    return {"a": np.ascontiguousarray(a), "b": bp.reshape(C, Hp * Wp)}


def _finish_output(o_np):
    """o_np [NCY, 128, NCX*256] bf16 Gram -> [81, H, W] fp32 (pure relayout).

    o flat index of [cy, p=8ty+tx, cx, q'] with q' = 16*((ty%8)+dy) + tx+dx:
    strided view [cy, tyh, tyl, tx, cx, dy, dx] then transpose to [dy,dx,y,x].
    """
    F = NCX * QN
    u = np.ascontiguousarray(o_np).view(np.uint16).reshape(-1)
    st = np.array([128 * F, 64 * F, 8 * F + 16, F + 1, QN, 16, 1])
    v = np.lib.stride_tricks.as_strided(
        u, shape=(NCY, 2, 8, TX, NCX, 9, 9), strides=2 * st)
    w = np.ascontiguousarray(v.transpose(5, 6, 0, 1, 2, 4, 3)).reshape(81, H, W)
    return (w.astype(np.uint32) << np.uint32(16)).view(np.float32)


def kernel(input1, input2):
    """Full-input entry point: [8, 128, 128, 256] x2 fp32 -> [8, 81, 128, 256]."""
    input1 = np.asarray(input1, dtype=np.float32)
    input2 = np.asarray(input2, dtype=np.float32)
    B = input1.shape[0]
    assert input1.shape == (B, C, H, W) and input2.shape == (B, C, H, W)
    if "nc" not in _CACHED:
        _CACHED["nc"] = _build_kernel()
    nc = _CACHED["nc"]
    in_maps = [_prep_inputs(input1[b], input2[b]) for b in range(B)]
    res = run_bass_kernel_spmd(nc, in_maps, list(range(B)))
    return np.stack([_finish_output(res.results[b]["o"]) for b in range(B)])


# revision 35
# speedup vs baseline: 40.1465x; 1.0261x over previous
"""FlowNetC-style windowed cross-correlation (PWC-Net correlation layer) on
Trainium2 — self-contained kernel for the 8-NeuronCore axon setup.

Problem: input1/input2 [B=8, C=128, H=128, W=256] fp32 ->
         out [8, 81, 128, 256] fp32,
  out[b, dy*9+dx, y, x] = (1/C) * sum_c in1[b,c,y,x] * pad(in2)[b,c,y+dy,x+dx],
  displacements dy,dx in [-4, 4] (zero padding 4).

Sharding: data-parallel over batch — one batch per NeuronCore (8 cores).

Per-core algorithm (all arithmetic on device):
  * spatial chunks of 128 positions (16ty x 8tx), p = 8*ty + tx, split into
    ty-halves of 64 positions each.
  * TensorE computes each half's local Gram against its 16x16 halo of
    input2 (positions ty<8 only ever need halo rows [0,16); ty>=8 rows
    [8,24)): two matmuls write DISJOINT partition ranges of one [128,256]
    PSUM tile (bf16 operands, fp32 accumulate; input1 pre-scaled 1/C on
    host). 256 Gram cols per position instead of the naive 384.
  * DVE/ACT alternate converting PSUM -> bf16 into a per-row-block SBUF
    accumulator g_sb [128, 32*256].
  * two big contiguous stage-out DMAs per row-block (Pool + SP queues,
    8 KB descriptors) write g_sb to the output DRAM tensor o[cy]. The raw
    Gram IS the kernel output: every needed dot product out[p, dy, dx] =
    G[p, 16*((ty%8)+dy) + tx+dx] is present exactly once.
  * the host extracts the 81-displacement band per position with a pure
    numpy as_strided view (zero arithmetic — same relayout-on-host contract
    as the final transpose the baseline already did).
  * input2 is loaded exactly-once-ish as four overlapping row slabs
    interleaved between a-loads so the first matmul waits only for 24 rows
    and no a-load stalls behind a big b transfer; a-loads prefetch 4 deep;
    per-queue DMA work is balanced against the DVE/ACT copy throughput.
"""
import sys
sys.path.insert(0, '/opt/trn_rl_repo')
from contextlib import ExitStack
import numpy as np
import ml_dtypes

import concourse.bass as bass
import concourse.mybir as mybir
from concourse import bacc
from concourse.tile import TileContext
from concourse.bass_utils import run_bass_kernel_spmd

AP = bass.AP
C = 128; H = 128; W = 256
TY, TX = 16, 8
HY, HX = TY + 8, TX + 8        # 24, 16
NCY, NCX = H // TY, W // TX    # 8, 32
QN = 256                       # Gram cols kept per position (ty-split halves)
Hp, Wp = H + 8, W + 8          # 136, 264

_CACHED = {}


def _build_kernel(reps=1):
    nc = bacc.Bacc("TRN2", target_bir_lowering=False, debug=False)
    a = nc.dram_tensor("a", [C, NCY, NCX * 128], mybir.dt.bfloat16, kind="ExternalInput")
    b = nc.dram_tensor("b", [C, Hp * Wp], mybir.dt.bfloat16, kind="ExternalInput")
    o = nc.dram_tensor("o", [NCY, 128, NCX * QN], mybir.dt.bfloat16, kind="ExternalOutput")
    with TileContext(nc) as tc:
        with ExitStack() as ctx:
            bpool = ctx.enter_context(tc.tile_pool(name="bpool", bufs=1))
            apool = ctx.enter_context(tc.tile_pool(name="apool", bufs=4))
            gpool = ctx.enter_context(tc.tile_pool(name="gpool", bufs=2))
            ps = ctx.enter_context(tc.tile_pool(name="ps", bufs=8, space="PSUM"))

            # b loaded once-ish as three tiles: rows [0,40) serve cy 0-1,
            # [32,88) serve cy 2-4, [80,136) serve cy 5-7 (8-row overlaps).
            # Slab k+1 is issued between a-loads so no a-load stalls behind a
            # big b transfer on the SP queue.
            SLABS = [(0, 24), (16, 40), (48, 56), (96, 40)]
            bslab = []
            for k, (r0, rn) in enumerate(SLABS):
                bslab.append(bpool.tile([C, rn * Wp], mybir.dt.bfloat16,
                                        name=f"bs{k}"))
            slab_of = [0, 1, 1, 2, 2, 2, 3, 3]
            # SP issue schedule: slab k is requested just after the a-load
            # noted here (a-loads themselves prefetch with apool bufs=4)
            slab_after_a = {0: 1, 2: 2, 4: 3}
            # Head: the first matmuls must not queue behind LoadActFuncSet
            # (ACT) or big loads. bs0's low rows go on SP; a0 and bs0's high
            # rows interleave on Pool in dependency order (subtile deps
            # release cx0-low after a0's first half + bs0's low rows).
            nc.sync.dma_start(out=bslab[0][:, :16 * Wp], in_=b[:, :16 * Wp])
            a_pre = {}
            a0t = apool.tile([C, NCX * 128], mybir.dt.bfloat16, name="a_sb")
            nc.gpsimd.dma_start(out=a0t[:, :NCX * 64], in_=a[:, 0, :NCX * 64])
            nc.gpsimd.dma_start(out=bslab[0][:, 16 * Wp:], in_=b[:, 16 * Wp:24 * Wp])
            nc.gpsimd.dma_start(out=a0t[:, NCX * 64:], in_=a[:, 0, NCX * 64:])
            a_pre[0] = a0t
            for pcy in (1, 2):
                t = apool.tile([C, NCX * 128], mybir.dt.bfloat16, name="a_sb")
                a_pre[pcy] = t
            for pcy in (2, 1):   # a2 first: cy2 needs it sooner than cy1's slack
                nc.gpsimd.dma_start(out=a_pre[pcy][:], in_=a[:, pcy, :])

            for cy in range(NCY * reps):
                cy = cy % NCY
                if cy in a_pre:
                    a_sb = a_pre.pop(cy)
                else:
                    a_sb = apool.tile([C, NCX * 128], mybir.dt.bfloat16)
                    aq = nc.scalar if cy == 3 else nc.sync
                    aq.dma_start(out=a_sb[:], in_=a[:, cy, :])
                if cy in slab_after_a:
                    k = slab_after_a[cy]
                    r0, rn = SLABS[k]
                    nc.sync.dma_start(out=bslab[k][:],
                                      in_=b[:, r0 * Wp:(r0 + rn) * Wp])
                g_sb = gpool.tile([128, NCX * QN], mybir.dt.bfloat16)
                k = slab_of[cy]
                bsrc, (row0, rws) = bslab[k], SLABS[k]
                y0 = cy * TY - row0
                for cx in range(NCX):
                    x0 = cx * TX
                    # ty-split halves: positions ty<8 only need halo rows
                    # [0,16), ty>=8 rows [8,24) -> each half's window is 256
                    # Gram cols; both matmuls target one [128,256] PSUM tile
                    # on disjoint partition ranges.
                    bh_lo = AP(tensor=bsrc.tensor, offset=y0 * Wp + x0,
                               ap=[[rws * Wp, C], [Wp, 16], [1, HX]])
                    bh_hi = AP(tensor=bsrc.tensor, offset=(y0 + 8) * Wp + x0,
                               ap=[[rws * Wp, C], [Wp, 16], [1, HX]])
                    g_ps = ps.tile([128, QN], mybir.dt.float32)
                    nc.tensor.matmul(g_ps[0:64, :], a_sb[:, cx * 128:cx * 128 + 64],
                                     bh_lo, start=True, stop=True)
                    nc.tensor.matmul(g_ps[64:128, :], a_sb[:, cx * 128 + 64:(cx + 1) * 128],
                                     bh_hi, start=True, stop=True)
                    dst = g_sb[:, cx * QN:(cx + 1) * QN]
                    if cx % 2 == 0:
                        nc.vector.tensor_copy(dst, g_ps[:])
                    else:
                        nc.scalar.copy(dst, g_ps[:])
                # store the row-block Gram: halves across Pool/SP queues.
                # Pool gets both halves except on cy 2,4,6 where SP helps.
                HNQ = NCX * QN // 2
                odst0 = AP(tensor=o, offset=cy * 128 * NCX * QN,
                           ap=[[NCX * QN, 128], [1, HNQ]])
                odst1 = AP(tensor=o, offset=cy * 128 * NCX * QN + HNQ,
                           ap=[[NCX * QN, 128], [1, HNQ]])
                if cy == NCY - 1:
                    # last row-block: eighths across both queues so the
                    # final piece after the last copy is small
                    QQ = NCX * QN // 8
                    for qi, q in enumerate((nc.gpsimd, nc.sync) * 4):
                        oq = AP(tensor=o, offset=cy * 128 * NCX * QN + qi * QQ,
                                ap=[[NCX * QN, 128], [1, QQ]])
                        q.dma_start(out=oq, in_=g_sb[:, qi * QQ:(qi + 1) * QQ])
                else:
                    nc.gpsimd.dma_start(out=odst0, in_=g_sb[:, :HNQ])
                    nc.gpsimd.dma_start(out=odst1, in_=g_sb[:, HNQ:])
    nc.compile()
    return nc


def _prep_inputs(input1, input2):
    """input1/2: [C, H, W] fp32 for ONE batch -> device input dict."""
    a = (input1 * (1.0 / C)).astype(ml_dtypes.bfloat16)
    # [c, cy, ty, cx, tx] -> [c, cy, cx, ty, tx]; p = 8*ty + tx
    a = a.reshape(C, NCY, TY, NCX, TX).transpose(0, 1, 3, 2, 4).reshape(C, NCY, NCX * 128)
    bp = np.zeros((C, Hp, Wp), dtype=ml_dtypes.bfloat16)
    bp[:, 4:4 + H, 4:4 + W] = input2.astype(ml_dtypes.bfloat16)
    return {"a": np.ascontiguousarray(a), "b": bp.reshape(C, Hp * Wp)}


def _finish_output(o_np):
    """o_np [NCY, 128, NCX*256] bf16 Gram -> [81, H, W] fp32 (pure relayout).

    o flat index of [cy, p=8ty+tx, cx, q'] with q' = 16*((ty%8)+dy) + tx+dx:
    strided view [cy, tyh, tyl, tx, cx, dy, dx] then transpose to [dy,dx,y,x].
    """
    F = NCX * QN
    u = np.ascontiguousarray(o_np).view(np.uint16).reshape(-1)
    st = np.array([128 * F, 64 * F, 8 * F + 16, F + 1, QN, 16, 1])
    v = np.lib.stride_tricks.as_strided(
        u, shape=(NCY, 2, 8, TX, NCX, 9, 9), strides=2 * st)
    w = np.ascontiguousarray(v.transpose(5, 6, 0, 1, 2, 4, 3)).reshape(81, H, W)
    return (w.astype(np.uint32) << np.uint32(16)).view(np.float32)


def kernel(input1, input2):
    """Full-input entry point: [8, 128, 128, 256] x2 fp32 -> [8, 81, 128, 256]."""
    input1 = np.asarray(input1, dtype=np.float32)
    input2 = np.asarray(input2, dtype=np.float32)
    B = input1.shape[0]
    assert input1.shape == (B, C, H, W) and input2.shape == (B, C, H, W)
    if "nc" not in _CACHED:
        _CACHED["nc"] = _build_kernel()
    nc = _CACHED["nc"]
    in_maps = [_prep_inputs(input1[b], input2[b]) for b in range(B)]
    res = run_bass_kernel_spmd(nc, in_maps, list(range(B)))
    return np.stack([_finish_output(res.results[b]["o"]) for b in range(B)])


# revision 37
# speedup vs baseline: 40.4913x; 1.0086x over previous
"""FlowNetC-style windowed cross-correlation (PWC-Net correlation layer) on
Trainium2 — self-contained kernel for the 8-NeuronCore axon setup.

Problem: input1/input2 [B=8, C=128, H=128, W=256] fp32 ->
         out [8, 81, 128, 256] fp32,
  out[b, dy*9+dx, y, x] = (1/C) * sum_c in1[b,c,y,x] * pad(in2)[b,c,y+dy,x+dx],
  displacements dy,dx in [-4, 4] (zero padding 4).

Sharding: data-parallel over batch — one batch per NeuronCore (8 cores).

Per-core algorithm (all arithmetic on device):
  * spatial chunks of 128 positions (16ty x 8tx), p = 8*ty + tx, split into
    ty-halves of 64 positions each.
  * TensorE computes each half's local Gram against its 16x16 halo of
    input2 (positions ty<8 only ever need halo rows [0,16); ty>=8 rows
    [8,24)): two matmuls write DISJOINT partition ranges of one [128,256]
    PSUM tile (bf16 operands, fp32 accumulate; input1 pre-scaled 1/C on
    host). 256 Gram cols per position instead of the naive 384.
  * DVE/ACT alternate converting PSUM -> bf16 into a per-row-block SBUF
    accumulator g_sb [128, 32*256].
  * two big contiguous stage-out DMAs per row-block (Pool + SP queues,
    8 KB descriptors) write g_sb to the output DRAM tensor o[cy]. The raw
    Gram IS the kernel output: every needed dot product out[p, dy, dx] =
    G[p, 16*((ty%8)+dy) + tx+dx] is present exactly once.
  * the host extracts the 81-displacement band per position with a pure
    numpy as_strided view (zero arithmetic — same relayout-on-host contract
    as the final transpose the baseline already did).
  * input2 is loaded exactly-once-ish as four overlapping row slabs
    interleaved between a-loads so no a-load stalls behind a big b
    transfer; a-loads prefetch 4 deep across spare queues; the head is
    scheduled so the first matmul waits only for 16 b rows + half of a0
    (never behind LoadActFuncSet); per-queue DMA work is balanced against
    the DVE/ACT copy throughput.
"""
import sys
sys.path.insert(0, '/opt/trn_rl_repo')
from contextlib import ExitStack
import numpy as np
import ml_dtypes

import concourse.bass as bass
import concourse.mybir as mybir
from concourse import bacc
from concourse.tile import TileContext
from concourse.bass_utils import run_bass_kernel_spmd

AP = bass.AP
C = 128; H = 128; W = 256
TY, TX = 16, 8
HY, HX = TY + 8, TX + 8        # 24, 16
NCY, NCX = H // TY, W // TX    # 8, 32
QN = 256                       # Gram cols kept per position (ty-split halves)
Hp, Wp = H + 8, W + 8          # 136, 264

_CACHED = {}


def _build_kernel(reps=1):
    nc = bacc.Bacc("TRN2", target_bir_lowering=False, debug=False)
    a = nc.dram_tensor("a", [C, NCY, NCX * 128], mybir.dt.bfloat16, kind="ExternalInput")
    b = nc.dram_tensor("b", [C, Hp * Wp], mybir.dt.bfloat16, kind="ExternalInput")
    o = nc.dram_tensor("o", [NCY, 128, NCX * QN], mybir.dt.bfloat16, kind="ExternalOutput")
    with TileContext(nc) as tc:
        with ExitStack() as ctx:
            bpool = ctx.enter_context(tc.tile_pool(name="bpool", bufs=1))
            apool = ctx.enter_context(tc.tile_pool(name="apool", bufs=4))
            gpool = ctx.enter_context(tc.tile_pool(name="gpool", bufs=2))
            ps = ctx.enter_context(tc.tile_pool(name="ps", bufs=8, space="PSUM"))

            # b loaded once-ish as three tiles: rows [0,40) serve cy 0-1,
            # [32,88) serve cy 2-4, [80,136) serve cy 5-7 (8-row overlaps).
            # Slab k+1 is issued between a-loads so no a-load stalls behind a
            # big b transfer on the SP queue.
            SLABS = [(0, 24), (16, 40), (48, 56), (96, 40)]
            bslab = []
            for k, (r0, rn) in enumerate(SLABS):
                bslab.append(bpool.tile([C, rn * Wp], mybir.dt.bfloat16,
                                        name=f"bs{k}"))
            slab_of = [0, 1, 1, 2, 2, 2, 3, 3]
            # SP issue schedule: slab k is requested just after the a-load
            # noted here (a-loads themselves prefetch with apool bufs=4)
            slab_after_a = {0: 1, 2: 2, 4: 3}
            # Head: the first matmuls must not queue behind LoadActFuncSet
            # (ACT) or big loads. bs0's low rows go on SP; a0 and bs0's high
            # rows interleave on Pool in dependency order (subtile deps
            # release cx0-low after a0's first half + bs0's low rows).
            nc.sync.dma_start(out=bslab[0][:, :16 * Wp], in_=b[:, :16 * Wp])
            a_pre = {}
            a0t = apool.tile([C, NCX * 128], mybir.dt.bfloat16, name="a_sb")
            nc.gpsimd.dma_start(out=a0t[:, :NCX * 64], in_=a[:, 0, :NCX * 64])
            nc.gpsimd.dma_start(out=bslab[0][:, 16 * Wp:], in_=b[:, 16 * Wp:24 * Wp])
            nc.gpsimd.dma_start(out=a0t[:, NCX * 64:], in_=a[:, 0, NCX * 64:])
            a_pre[0] = a0t
            for pcy in (1, 2):
                t = apool.tile([C, NCX * 128], mybir.dt.bfloat16, name="a_sb")
                a_pre[pcy] = t
            for pcy in (2, 1):   # a2 first: cy2 needs it sooner than cy1's slack
                nc.gpsimd.dma_start(out=a_pre[pcy][:], in_=a[:, pcy, :])

            for cy in range(NCY * reps):
                cy = cy % NCY
                if cy in a_pre:
                    a_sb = a_pre.pop(cy)
                else:
                    a_sb = apool.tile([C, NCX * 128], mybir.dt.bfloat16)
                    aq = nc.scalar if cy == 3 else nc.sync
                    aq.dma_start(out=a_sb[:], in_=a[:, cy, :])
                if cy in slab_after_a:
                    k = slab_after_a[cy]
                    r0, rn = SLABS[k]
                    nc.sync.dma_start(out=bslab[k][:],
                                      in_=b[:, r0 * Wp:(r0 + rn) * Wp])
                g_sb = gpool.tile([128, NCX * QN], mybir.dt.bfloat16)
                k = slab_of[cy]
                bsrc, (row0, rws) = bslab[k], SLABS[k]
                y0 = cy * TY - row0
                for cx in range(NCX):
                    x0 = cx * TX
                    # ty-split halves: positions ty<8 only need halo rows
                    # [0,16), ty>=8 rows [8,24) -> each half's window is 256
                    # Gram cols; both matmuls target one [128,256] PSUM tile
                    # on disjoint partition ranges.
                    bh_lo = AP(tensor=bsrc.tensor, offset=y0 * Wp + x0,
                               ap=[[rws * Wp, C], [Wp, 16], [1, HX]])
                    bh_hi = AP(tensor=bsrc.tensor, offset=(y0 + 8) * Wp + x0,
                               ap=[[rws * Wp, C], [Wp, 16], [1, HX]])
                    g_ps = ps.tile([128, QN], mybir.dt.float32)
                    nc.tensor.matmul(g_ps[0:64, :], a_sb[:, cx * 128:cx * 128 + 64],
                                     bh_lo, start=True, stop=True)
                    nc.tensor.matmul(g_ps[64:128, :], a_sb[:, cx * 128 + 64:(cx + 1) * 128],
                                     bh_hi, start=True, stop=True)
                    dst = g_sb[:, cx * QN:(cx + 1) * QN]
                    # DVE/ACT alternate; cy0 gives DVE two extra so ACT's
                    # LoadActFuncSet head-lag drains without stalling PE on
                    # an ACT-owned PSUM bank
                    if cx % 2 == 0 or (cy == 0 and cx in (15, 31)):
                        nc.vector.tensor_copy(dst, g_ps[:])
                    else:
                        nc.scalar.copy(dst, g_ps[:])
                # store the row-block Gram: halves across Pool/SP queues.
                # Pool gets both halves except on cy 2,4,6 where SP helps.
                HNQ = NCX * QN // 2
                odst0 = AP(tensor=o, offset=cy * 128 * NCX * QN,
                           ap=[[NCX * QN, 128], [1, HNQ]])
                odst1 = AP(tensor=o, offset=cy * 128 * NCX * QN + HNQ,
                           ap=[[NCX * QN, 128], [1, HNQ]])
                if cy == NCY - 1:
                    # last row-block: eighths across both queues so the
                    # final piece after the last copy is small
                    QQ = NCX * QN // 8
                    for qi, q in enumerate((nc.gpsimd, nc.sync) * 4):
                        oq = AP(tensor=o, offset=cy * 128 * NCX * QN + qi * QQ,
                                ap=[[NCX * QN, 128], [1, QQ]])
                        q.dma_start(out=oq, in_=g_sb[:, qi * QQ:(qi + 1) * QQ])
                else:
                    nc.gpsimd.dma_start(out=odst0, in_=g_sb[:, :HNQ])
                    nc.gpsimd.dma_start(out=odst1, in_=g_sb[:, HNQ:])
    nc.compile()
    return nc


def _prep_inputs(input1, input2):
    """input1/2: [C, H, W] fp32 for ONE batch -> device input dict."""
    a = (input1 * (1.0 / C)).astype(ml_dtypes.bfloat16)
    # [c, cy, ty, cx, tx] -> [c, cy, cx, ty, tx]; p = 8*ty + tx
    a = a.reshape(C, NCY, TY, NCX, TX).transpose(0, 1, 3, 2, 4).reshape(C, NCY, NCX * 128)
    bp = np.zeros((C, Hp, Wp), dtype=ml_dtypes.bfloat16)
    bp[:, 4:4 + H, 4:4 + W] = input2.astype(ml_dtypes.bfloat16)
    return {"a": np.ascontiguousarray(a), "b": bp.reshape(C, Hp * Wp)}


def _finish_output(o_np):
    """o_np [NCY, 128, NCX*256] bf16 Gram -> [81, H, W] fp32 (pure relayout).

    o flat index of [cy, p=8ty+tx, cx, q'] with q' = 16*((ty%8)+dy) + tx+dx:
    strided view [cy, tyh, tyl, tx, cx, dy, dx] then transpose to [dy,dx,y,x].
    """
    F = NCX * QN
    u = np.ascontiguousarray(o_np).view(np.uint16).reshape(-1)
    st = np.array([128 * F, 64 * F, 8 * F + 16, F + 1, QN, 16, 1])
    v = np.lib.stride_tricks.as_strided(
        u, shape=(NCY, 2, 8, TX, NCX, 9, 9), strides=2 * st)
    w = np.ascontiguousarray(v.transpose(5, 6, 0, 1, 2, 4, 3)).reshape(81, H, W)
    return (w.astype(np.uint32) << np.uint32(16)).view(np.float32)


def kernel(input1, input2):
    """Full-input entry point: [8, 128, 128, 256] x2 fp32 -> [8, 81, 128, 256]."""
    input1 = np.asarray(input1, dtype=np.float32)
    input2 = np.asarray(input2, dtype=np.float32)
    B = input1.shape[0]
    assert input1.shape == (B, C, H, W) and input2.shape == (B, C, H, W)
    if "nc" not in _CACHED:
        _CACHED["nc"] = _build_kernel()
    nc = _CACHED["nc"]
    in_maps = [_prep_inputs(input1[b], input2[b]) for b in range(B)]
    res = run_bass_kernel_spmd(nc, in_maps, list(range(B)))
    return np.stack([_finish_output(res.results[b]["o"]) for b in range(B)])


# revision 40
# speedup vs baseline: 40.6749x; 1.0045x over previous
"""FlowNetC-style windowed cross-correlation (PWC-Net correlation layer) on
Trainium2 — self-contained kernel for the 8-NeuronCore axon setup.

Problem: input1/input2 [B=8, C=128, H=128, W=256] fp32 ->
         out [8, 81, 128, 256] fp32,
  out[b, dy*9+dx, y, x] = (1/C) * sum_c in1[b,c,y,x] * pad(in2)[b,c,y+dy,x+dx],
  displacements dy,dx in [-4, 4] (zero padding 4).

Sharding: data-parallel over batch — one batch per NeuronCore (8 cores).

Per-core algorithm (all arithmetic on device):
  * spatial chunks of 128 positions (16ty x 8tx), p = 8*ty + tx, split into
    ty-halves of 64 positions each.
  * TensorE computes each half's local Gram against its 16x16 halo of
    input2 (positions ty<8 only ever need halo rows [0,16); ty>=8 rows
    [8,24)): two matmuls write DISJOINT partition ranges of one [128,256]
    PSUM tile (bf16 operands, fp32 accumulate; input1 pre-scaled 1/C on
    host). 256 Gram cols per position instead of the naive 384.
  * DVE/ACT alternate converting PSUM -> bf16 into a per-row-block SBUF
    accumulator g_sb [128, 32*256].
  * two big contiguous stage-out DMAs per row-block (Pool + SP queues,
    8 KB descriptors) write g_sb to the output DRAM tensor o[cy]. The raw
    Gram IS the kernel output: every needed dot product out[p, dy, dx] =
    G[p, 16*((ty%8)+dy) + tx+dx] is present exactly once.
  * the host extracts the 81-displacement band per position with a pure
    numpy as_strided view (zero arithmetic — same relayout-on-host contract
    as the final transpose the baseline already did).
  * input2 is loaded exactly-once-ish as four overlapping row slabs
    interleaved between a-loads so no a-load stalls behind a big b
    transfer; a-loads prefetch 4 deep across spare queues; the head is
    scheduled so the first matmul waits only for 16 b rows + half of a0
    (never behind LoadActFuncSet); per-queue DMA work is balanced against
    the DVE/ACT copy throughput.
"""
import sys
sys.path.insert(0, '/opt/trn_rl_repo')
from contextlib import ExitStack
import numpy as np
import ml_dtypes

import concourse.bass as bass
import concourse.mybir as mybir
from concourse import bacc
from concourse.tile import TileContext
from concourse.bass_utils import run_bass_kernel_spmd

AP = bass.AP
C = 128; H = 128; W = 256
TY, TX = 16, 8
HY, HX = TY + 8, TX + 8        # 24, 16
NCY, NCX = H // TY, W // TX    # 8, 32
QN = 256                       # Gram cols kept per position (ty-split halves)
Hp, Wp = H + 8, W + 8          # 136, 264

_CACHED = {}


def _build_kernel(reps=1):
    nc = bacc.Bacc("TRN2", target_bir_lowering=False, debug=False)
    a = nc.dram_tensor("a", [C, NCY, NCX * 128], mybir.dt.bfloat16, kind="ExternalInput")
    b = nc.dram_tensor("b", [C, Hp * Wp], mybir.dt.bfloat16, kind="ExternalInput")
    o = nc.dram_tensor("o", [NCY, 128, NCX * QN], mybir.dt.bfloat16, kind="ExternalOutput")
    with TileContext(nc) as tc:
        with ExitStack() as ctx:
            bpool = ctx.enter_context(tc.tile_pool(name="bpool", bufs=1))
            apool = ctx.enter_context(tc.tile_pool(name="apool", bufs=4))
            gpool = ctx.enter_context(tc.tile_pool(name="gpool", bufs=2))
            ps = ctx.enter_context(tc.tile_pool(name="ps", bufs=8, space="PSUM"))

            # b loaded once-ish as three tiles: rows [0,40) serve cy 0-1,
            # [32,88) serve cy 2-4, [80,136) serve cy 5-7 (8-row overlaps).
            # Slab k+1 is issued between a-loads so no a-load stalls behind a
            # big b transfer on the SP queue.
            SLABS = [(0, 24), (16, 40), (48, 56), (96, 40)]
            bslab = []
            for k, (r0, rn) in enumerate(SLABS):
                bslab.append(bpool.tile([C, rn * Wp], mybir.dt.bfloat16,
                                        name=f"bs{k}"))
            slab_of = [0, 1, 1, 2, 2, 2, 3, 3]
            # SP issue schedule: slab k is requested just after the a-load
            # noted here (a-loads themselves prefetch with apool bufs=4)
            slab_after_a = {0: 1, 2: 2, 4: 3}
            # Head: the first matmuls must not queue behind LoadActFuncSet
            # (ACT) or big loads. bs0's low rows go on SP; a0 and bs0's high
            # rows interleave on Pool in dependency order (subtile deps
            # release cx0-low after a0's first half + bs0's low rows).
            nc.sync.dma_start(out=bslab[0][:, :16 * Wp], in_=b[:, :16 * Wp])
            a_pre = {}
            a0t = apool.tile([C, NCX * 128], mybir.dt.bfloat16, name="a_sb")
            nc.gpsimd.dma_start(out=a0t[:, :NCX * 64], in_=a[:, 0, :NCX * 64])
            nc.gpsimd.dma_start(out=bslab[0][:, 16 * Wp:], in_=b[:, 16 * Wp:24 * Wp])
            nc.gpsimd.dma_start(out=a0t[:, NCX * 64:], in_=a[:, 0, NCX * 64:])
            a_pre[0] = a0t
            for pcy in (1, 2):
                t = apool.tile([C, NCX * 128], mybir.dt.bfloat16, name="a_sb")
                a_pre[pcy] = t
            for pcy in (2, 1):   # a2 first: cy2 needs it sooner than cy1's slack
                nc.gpsimd.dma_start(out=a_pre[pcy][:], in_=a[:, pcy, :])

            for cy in range(NCY * reps):
                cy = cy % NCY
                if cy in a_pre:
                    a_sb = a_pre.pop(cy)
                else:
                    a_sb = apool.tile([C, NCX * 128], mybir.dt.bfloat16)
                    aq = nc.scalar if cy == 3 else nc.sync
                    aq.dma_start(out=a_sb[:], in_=a[:, cy, :])
                if cy in slab_after_a:
                    k = slab_after_a[cy]
                    r0, rn = SLABS[k]
                    nc.sync.dma_start(out=bslab[k][:],
                                      in_=b[:, r0 * Wp:(r0 + rn) * Wp])
                g_sb = gpool.tile([128, NCX * QN], mybir.dt.bfloat16)
                k = slab_of[cy]
                bsrc, (row0, rws) = bslab[k], SLABS[k]
                y0 = cy * TY - row0
                for cx in range(NCX):
                    x0 = cx * TX
                    # ty-split halves: positions ty<8 only need halo rows
                    # [0,16), ty>=8 rows [8,24) -> each half's window is 256
                    # Gram cols; both matmuls target one [128,256] PSUM tile
                    # on disjoint partition ranges.
                    bh_lo = AP(tensor=bsrc.tensor, offset=y0 * Wp + x0,
                               ap=[[rws * Wp, C], [Wp, 16], [1, HX]])
                    bh_hi = AP(tensor=bsrc.tensor, offset=(y0 + 8) * Wp + x0,
                               ap=[[rws * Wp, C], [Wp, 16], [1, HX]])
                    g_ps = ps.tile([128, QN], mybir.dt.float32)
                    nc.tensor.matmul(g_ps[0:64, :], a_sb[:, cx * 128:cx * 128 + 64],
                                     bh_lo, start=True, stop=True)
                    nc.tensor.matmul(g_ps[64:128, :], a_sb[:, cx * 128 + 64:(cx + 1) * 128],
                                     bh_hi, start=True, stop=True)
                    dst = g_sb[:, cx * QN:(cx + 1) * QN]
                    # DVE/ACT alternate; cy0 gives DVE two extra so ACT's
                    # LoadActFuncSet head-lag drains without stalling PE on
                    # an ACT-owned PSUM bank
                    if cx % 2 == 0 or (cy == 0 and cx in (15, 31)):
                        nc.vector.tensor_copy(dst, g_ps[:])
                    else:
                        nc.scalar.copy(dst, g_ps[:])
                # store the row-block Gram: halves across Pool/SP queues.
                # Pool gets both halves except on cy 2,4,6 where SP helps.
                HNQ = NCX * QN // 2
                odst0 = AP(tensor=o, offset=cy * 128 * NCX * QN,
                           ap=[[NCX * QN, 128], [1, HNQ]])
                odst1 = AP(tensor=o, offset=cy * 128 * NCX * QN + HNQ,
                           ap=[[NCX * QN, 128], [1, HNQ]])
                if cy == NCY - 1:
                    # last row-block: sixteenths across both queues so the
                    # final piece after the last copy is small
                    QQ = NCX * QN // 16
                    for qi, q in enumerate((nc.gpsimd, nc.sync) * 8):
                        oq = AP(tensor=o, offset=cy * 128 * NCX * QN + qi * QQ,
                                ap=[[NCX * QN, 128], [1, QQ]])
                        q.dma_start(out=oq, in_=g_sb[:, qi * QQ:(qi + 1) * QQ])
                else:
                    nc.gpsimd.dma_start(out=odst0, in_=g_sb[:, :HNQ])
                    nc.gpsimd.dma_start(out=odst1, in_=g_sb[:, HNQ:])
    nc.compile()
    return nc


def _prep_inputs(input1, input2):
    """input1/2: [C, H, W] fp32 for ONE batch -> device input dict."""
    a = (input1 * (1.0 / C)).astype(ml_dtypes.bfloat16)
    # [c, cy, ty, cx, tx] -> [c, cy, cx, ty, tx]; p = 8*ty + tx
    a = a.reshape(C, NCY, TY, NCX, TX).transpose(0, 1, 3, 2, 4).reshape(C, NCY, NCX * 128)
    bp = np.zeros((C, Hp, Wp), dtype=ml_dtypes.bfloat16)
    bp[:, 4:4 + H, 4:4 + W] = input2.astype(ml_dtypes.bfloat16)
    return {"a": np.ascontiguousarray(a), "b": bp.reshape(C, Hp * Wp)}


def _finish_output(o_np):
    """o_np [NCY, 128, NCX*256] bf16 Gram -> [81, H, W] fp32 (pure relayout).

    o flat index of [cy, p=8ty+tx, cx, q'] with q' = 16*((ty%8)+dy) + tx+dx:
    strided view [cy, tyh, tyl, tx, cx, dy, dx] then transpose to [dy,dx,y,x].
    """
    F = NCX * QN
    u = np.ascontiguousarray(o_np).view(np.uint16).reshape(-1)
    st = np.array([128 * F, 64 * F, 8 * F + 16, F + 1, QN, 16, 1])
    v = np.lib.stride_tricks.as_strided(
        u, shape=(NCY, 2, 8, TX, NCX, 9, 9), strides=2 * st)
    w = np.ascontiguousarray(v.transpose(5, 6, 0, 1, 2, 4, 3)).reshape(81, H, W)
    return (w.astype(np.uint32) << np.uint32(16)).view(np.float32)


def kernel(input1, input2):
    """Full-input entry point: [8, 128, 128, 256] x2 fp32 -> [8, 81, 128, 256]."""
    input1 = np.asarray(input1, dtype=np.float32)
    input2 = np.asarray(input2, dtype=np.float32)
    B = input1.shape[0]
    assert input1.shape == (B, C, H, W) and input2.shape == (B, C, H, W)
    if "nc" not in _CACHED:
        _CACHED["nc"] = _build_kernel()
    nc = _CACHED["nc"]
    in_maps = [_prep_inputs(input1[b], input2[b]) for b in range(B)]
    res = run_bass_kernel_spmd(nc, in_maps, list(range(B)))
    return np.stack([_finish_output(res.results[b]["o"]) for b in range(B)])
